# revision 20
# baseline (speedup 1.0000x reference)
"""Trainium2 Bass kernel: multi-head attention (dense transformer block).

Computation (per batch b):
    Q = x @ Wq + bq ; K = x @ Wk + bk ; V = x @ Wv + bv        (per head)
    P = exp((Q @ K^T) / sqrt(Dh))                   (no max-subtraction needed:
                                                     scores are O(1) by construction)
    out = sum_h (P @ V / rowsum(P)) @ Wd[h] + bd

Sharding (data + tensor parallel): 8 cores; core c handles batch b = c // 4
and the 4 heads starting at 4*(c % 4). Each core computes a partial [L, D]
output (bf16); the host sums the 4 partials per batch and adds bd.

Host-side sharding picks the on-device layout: x is pre-transposed to
x^T [dmodel, L] and cast to bf16, and the weights are pre-packed bf16 in
exactly the SBUF layout the kernel wants. The device then needs NO
transposes and NO dtype casts on the input path -- plain contiguous DMAs.

Per-core dataflow (one SPMD program):
  - A few warm-up matmuls on junk data keep the PE HAM clock at 2.4 GHz
    before the first projection arrives.
  - Heads processed in 2 pairs stacked on partitions. K^T/Q^T/V^T
    [128=2x64, L] bf16; scores S^T = K^T.T @ Q^T land with the softmax axis
    on PSUM partitions; each "beat" is one l'-tile with the two heads'
    matmuls adjacent so they run on independent 64-row PE tiles
    concurrently, and ONE exp instruction covers both heads.
  - Work is chunked "sub-major" (512 query positions at a time): scores ->
    exp -> attend -> normalize per 512-l sub-chunk, so attend trails the exp
    stream by only one sub-chunk.
  - exp on ScalarE ONLY (nothing else runs there): [128, 1024] per
    instruction from PSUM, bf16 out. This is the wall: ~171us measured.
  - V is projected as V^T (cheap, weights stationary) and PE-transposed
    per l-tile into vt [l', 1|V_h0|1|V_h1]; the ones columns make attend
    PSUM rows 0..63 the softmax denominator for free, at base partition 0
    where reciprocal_approx_fast needs it.
  - normalize: two small DVE copies + reciprocal_approx_fast, multiply on
    GPSIMD (which cannot touch PSUM).
  - out-proj per l-tile as soon as both pairs' o_norm is ready; y bf16.
Emission is software-pipelined: V / attend / K / out-proj work is
interleaved between score beats as PE filler so ScalarE's exp stream never
starves and the PE never head-of-line blocks.
"""

import os
import sys
from contextlib import ExitStack

import ml_dtypes
import numpy as np

for _p in ("/opt/trn_rl_repo", "/root/.axon_site/_ro/trn_rl_repo"):
    if os.path.isdir(_p) and _p not in sys.path:
        sys.path.append(_p)

import concourse.bass as bass
import concourse.tile as tile
from concourse import bacc, mybir
from concourse.bass import ds
from concourse.bass_utils import run_bass_kernel_spmd
from concourse.masks import make_identity

F32 = mybir.dt.float32
BF16 = mybir.dt.bfloat16
BF16_NP = ml_dtypes.bfloat16

# Problem sizes (hardcoded per contract).
DMODEL, HEADS, DHEAD = 1024, 16, 64
B, L = 2, 2048
NCORES = 8
H_PER_CORE = B * HEADS // NCORES          # 4 heads per core
NPAIR = H_PER_CORE // 2                   # head pairs per core
P = 128                                   # partitions
KT = DMODEL // P                          # 8 k-tiles over dmodel
NLT = L // P                              # 16 l-tiles
LCH = 512                                 # sub-chunk width (one psum bank)
NSUB = L // LCH                           # 4 sub-chunks
MCH = 512                                 # m-chunk for out-proj
NMC = DMODEL // MCH
PT_BUFS = 48                              # P^T tiles in flight (3 sub-chunks)
WCH = 3 * KT * P                          # per-pair packed qkv weight columns


def build_nc():
    """Build the SPMD Bass program for one core."""
    nc = bacc.Bacc("TRN2", target_bir_lowering=False, debug=False,
                   num_devices=NCORES)

    # Pre-packed per-core inputs (see shard_inputs):
    #   x:    x^T bf16 [dmodel, L]
    #   wqkv: bf16 [128, pp(2) * i(3: K,Q,V) * kt(8) * 128]
    #   wd2:  bf16 [128, pp(2) * dmodel]
    #   biasp: f32 [128, i(3: K,Q,V) * pp(2)]
    #   bv:   f32 [256] (flat, for the free-axis broadcast)
    x_d = nc.dram_tensor("x", [P, NSUB * KT * LCH], BF16, kind="ExternalInput").ap()
    wqkv_d = nc.dram_tensor("wqkv", [P, NPAIR * WCH], BF16, kind="ExternalInput").ap()
    wd2_d = nc.dram_tensor("wd2", [P, NPAIR * DMODEL], BF16, kind="ExternalInput").ap()
    biasp_d = nc.dram_tensor("biasp", [P, 3 * NPAIR], F32, kind="ExternalInput").ap()
    y_d = nc.dram_tensor("y", [L, DMODEL], BF16, kind="ExternalOutput").ap()

    with ExitStack() as ctx:
        tc = ctx.enter_context(tile.TileContext(nc))
        _body(nc, tc, ctx, x_d, wqkv_d, wd2_d, biasp_d, y_d)
    nc.compile()
    return nc


def _body(nc, tc, ctx, x_d, wqkv_d, wd2_d, biasp_d, y_d):
    const = ctx.enter_context(tc.tile_pool(name="const", bufs=1))
    sb = ctx.enter_context(tc.tile_pool(name="sb", bufs=1))
    psum = ctx.enter_context(tc.tile_pool(name="psum", bufs=1, space="PSUM"))

    # ACT exp-table warmup off the critical path
    warm = const.tile([P, 1], BF16)
    zz = const.tile([P, 1], F32)
    nc.vector.memset(zz, 0.0)
    nc.scalar.activation(warm, zz, func=mybir.ActivationFunctionType.Exp)

    # PE identity (for V transposes) + HAM warm-up matmuls on junk data:
    # keeps the PE clock at 2.4 GHz so the first projections don't run at
    # half rate.
    ident = const.tile([P, P], BF16)
    make_identity(nc, ident)
    for _ in range(13):
        jp = psum.tile([P, 2, P], F32, tag="qkvp", bufs=2, name="jp")
        nc.tensor.matmul(jp[:, 0], lhsT=ident, rhs=ident, start=True, stop=True)
        nc.tensor.matmul(jp[:, 1], lhsT=ident, rhs=ident, start=True, stop=True)

    # ---- input DMAs: weights on the scalar HWDGE queue, x on sync ----
    w_sb = const.tile([P, NPAIR, 3, KT, P], BF16)    # [k, pp, KQV, kt, n]
    wd_sb = const.tile([P, NPAIR, DMODEL], BF16)
    wf = w_sb.rearrange("k pp i kt n -> k (pp i kt n)")
    nc.scalar.dma_start(wf[:, 0:KT * P], wqkv_d[:, 0:KT * P])      # K pair 0
    xt = sb.tile([P, NSUB, KT, LCH], BF16)
    xtf = xt.rearrange("k c kt l -> k (c kt l)")
    CB = KT * LCH
    nc.sync.dma_start(xtf[:, ds(0, CB // 2)], x_d[:, ds(0, CB // 2)])
    nc.sync.dma_start(xtf[:, ds(CB // 2, CB // 2)], x_d[:, ds(CB // 2, CB // 2)])
    nc.scalar.dma_start(wf[:, KT * P:WCH], wqkv_d[:, KT * P:WCH])  # QV pair 0
    nc.sync.dma_start(xtf[:, ds(CB, CB)], x_d[:, ds(CB, CB)])
    nc.scalar.dma_start(wf[:, WCH:], wqkv_d[:, WCH:])              # pair 1
    nc.sync.dma_start(xtf[:, ds(2 * CB, CB)], x_d[:, ds(2 * CB, CB)])
    nc.scalar.dma_start(wd_sb.rearrange("k pp m -> k (pp m)"), wd2_d)
    nc.sync.dma_start(xtf[:, ds(3 * CB, CB)], x_d[:, ds(3 * CB, CB)])

    bias_sb = const.tile([P, 3, NPAIR], F32)
    nc.gpsimd.dma_start(bias_sb.rearrange("k i p -> k (i p)"), biasp_d)

    o_norm = sb.tile([P, NPAIR, L], BF16)

    def qkv_chunk(dst, p, i, lc):
        """dst[:, lc*LCH:+LCH] = (W_i^T x^T + b_i) in bf16. i: 0=K, 1=Q, 2=V."""
        ps = psum.tile([P, LCH], F32, tag="qkvp", bufs=2, name="qkvps")
        for kt in range(KT):
            nc.tensor.matmul(
                ps, lhsT=w_sb[:, p, i, kt],
                rhs=xt[:, lc, kt, :],
                start=(kt == 0), stop=(kt == KT - 1))
        nc.vector.tensor_scalar_add(
            dst[:, ds(lc * LCH, LCH)], ps, bias_sb[:, i, p:p + 1])

    def qkv_chunk2(dst, p, i, lc0):
        """Two adjacent chunks with one weight load per kt (interleaved
        accumulation groups on the two qkvp banks)."""
        psa = psum.tile([P, LCH], F32, tag="qkvp", bufs=2, name="qkvpa")
        psb = psum.tile([P, LCH], F32, tag="qkvp", bufs=2, name="qkvpb")
        for kt in range(KT):
            for ps, lc in ((psa, lc0), (psb, lc0 + 1)):
                nc.tensor.matmul(
                    ps, lhsT=w_sb[:, p, i, kt],
                    rhs=xt[:, lc, kt, :],
                    start=(kt == 0), stop=(kt == KT - 1),
                    skip_group_check=True)
        for ps, lc in ((psa, lc0), (psb, lc0 + 1)):
            nc.vector.tensor_scalar_add(
                dst[:, ds(lc * LCH, LCH)], ps, bias_sb[:, i, p:p + 1])

    # per-pair state
    kT_sb = [None] * NPAIR
    qT = [None] * NPAIR
    vT = [None] * NPAIR
    vt = [None] * NPAIR
    # pt[p][s][lt]: [128, 2(h), 512] bf16
    pt_tiles = [[[None] * NLT for _ in range(NSUB)] for _ in range(NPAIR)]

    def emit_vtr(p, lts):
        """PE-transpose V^T l-tiles into vt[:, lt] = [1*64|V_h0|1*64|V_h1]."""
        for lt in lts:
            tp = psum.tile([P, P], BF16, tag="qkvp", bufs=2, name="tp")
            nc.tensor.transpose(tp, vT[p][:, ds(lt * P, P)], ident)
            out3 = vt[p][:, lt].rearrange("p (i n) -> p i n", n=P)[:, :, DHEAD:P]
            nc.vector.tensor_copy(
                out3, tp.rearrange("p (i n) -> p i n", n=DHEAD))

    def emit_scores_exp(p, s, fillers):
        """16 beats (one l'-tile each, both heads) of scores + exp."""
        for lt in range(NLT):
            sp = psum.tile([P, 2, LCH], F32, tag="sctr", bufs=2, name="sp")
            for h in range(2):
                nc.tensor.matmul(
                    sp[:, h],
                    lhsT=kT_sb[p][ds(64 * h, 64), ds(lt * P, P)],
                    rhs=qT[p][ds(64 * h, 64), ds(s * LCH, LCH)],
                    start=True, stop=True)
            pt = sb.tile([P, 2, LCH], BF16, tag="pt", bufs=PT_BUFS, name="pt")
            nc.scalar.activation(
                pt.rearrange("p a b -> p (a b)"),
                sp.rearrange("p a b -> p (a b)"),
                func=mybir.ActivationFunctionType.Exp,
                scale=1.0 / np.sqrt(DHEAD))
            pt_tiles[p][s][lt] = pt
            if lt < len(fillers) and fillers[lt] is not None:
                fillers[lt]()

    att_op = {}

    N_P1 = 8

    def emit_attend_p1(p, s, h):
        """First part (l'-tiles 0..N_P1-1) of the attend accumulation."""
        op = psum.tile([P, LCH], F32, tag="op", bufs=2)
        att_op[(p, s, h)] = op
        for lt in range(N_P1):
            nc.tensor.matmul(
                op, lhsT=vt[p][:, lt, ds(P * h, P)],
                rhs=pt_tiles[p][s][lt][:, h, :],
                start=(lt == 0), stop=False, skip_group_check=True)

    def emit_attend(p, s, h):
        """O^T chunk = [1|V].T @ P^T; rows 0..63 are the denominator."""
        if (p, s, h) in att_op:
            op = att_op[(p, s, h)]
            lt0 = N_P1
        else:
            op = psum.tile([P, LCH], F32, tag="op", bufs=2)
            lt0 = 0
        for lt in range(lt0, NLT):
            nc.tensor.matmul(
                op, lhsT=vt[p][:, lt, ds(P * h, P)],
                rhs=pt_tiles[p][s][lt][:, h, :],
                start=(lt == 0), stop=(lt == NLT - 1),
                skip_group_check=True)
        _attend_norm(p, s, h, op)

    def emit_attend2(p, s0, h):
        """Attend for two subs with one weight load per l'-tile."""
        opa = psum.tile([P, LCH], F32, tag="op", bufs=2, name="opa")
        opb = psum.tile([P, LCH], F32, tag="op", bufs=2, name="opb")
        for lt in range(NLT):
            for op, s in ((opa, s0), (opb, s0 + 1)):
                nc.tensor.matmul(
                    op, lhsT=vt[p][:, lt, ds(P * h, P)],
                    rhs=pt_tiles[p][s][lt][:, h, :],
                    start=(lt == 0), stop=(lt == NLT - 1),
                    skip_group_check=True)
        _attend_norm(p, s0, h, opa)
        _attend_norm(p, s0 + 1, h, opb)

    def _attend_norm(p, s, h, op):
        dn = sb.tile([DHEAD, LCH], F32, tag="dn", bufs=2)
        nc.vector.tensor_copy(dn, op[0:DHEAD, :])
        rs = sb.tile([DHEAD, LCH], F32, tag="rs", bufs=2)
        nc.vector.reciprocal_approx_fast(rs, dn)
        osn = sb.tile([DHEAD, LCH], F32, tag="osn", bufs=2)
        nc.vector.tensor_copy(osn, op[DHEAD:P, :])
        nc.gpsimd.tensor_mul(
            o_norm[ds(64 * h, 64), p, ds(s * LCH, LCH)], osn, rs)

    def emit_outproj_lt(lt, wide=False):
        """Y[l-tile] = sum_pairs O^T.T @ Wd; bf16 out, one DMA per l-tile.
        wide=True borrows the (idle at tail) score psum banks for deeper
        pipelining."""
        ys = sb.tile([P, DMODEL], BF16, tag="ys", bufs=3)
        if wide:
            yp2 = psum.tile([P, 2, MCH], F32, tag="sctr", bufs=2, name="yp2")
            yps = [yp2[:, 0], yp2[:, 1]]
        else:
            yps = [psum.tile([P, MCH], F32, tag="qkvp", bufs=2, name="yp")
                   for _ in range(NMC)]
        for p in range(NPAIR):
            for mc in range(NMC):
                nc.tensor.matmul(
                    yps[mc], lhsT=o_norm[:, p, ds(lt * P, P)],
                    rhs=wd_sb[:, p, ds(mc * MCH, MCH)],
                    start=(p == 0), stop=(p == NPAIR - 1),
                    skip_group_check=True)
        for mc in range(NMC):
            nc.vector.tensor_copy(ys[:, ds(mc * MCH, MCH)], yps[mc])
        nc.sync.dma_start(y_d[ds(lt * P, P), :], ys)

    def F(fn, *a):
        return lambda: fn(*a)

    def emit_sub(p, s, fillers):
        if s == 0:
            qT[p] = sb.tile([P, L], BF16, tag="qT", bufs=1, name="qT")
        qkv_chunk(qT[p], p, 1, s)
        emit_scores_exp(p, s, fillers)

    def new_pair(p):
        vT[p] = sb.tile([P, L], BF16, tag="vT", bufs=1, name="vT")
        vt[p] = sb.tile([P, NLT, 2 * P], BF16, tag="vt", bufs=NPAIR, name="vt")
        nc.vector.memset(vt[p][:, :, 0:DHEAD], 1.0)
        nc.vector.memset(vt[p][:, :, P:P + DHEAD], 1.0)

    # ---- software-pipelined schedule (fillers are PE work placed between
    # score beats; cumulative PE work must stay ahead of the exp stream) ----
    kT_sb[0] = sb.tile([P, L], BF16, tag="kT", bufs=NPAIR, name="kT_sb")
    kT_sb[1] = sb.tile([P, L], BF16, tag="kT", bufs=NPAIR, name="kT_sb")
    new_pair(0)
    new_pair(1)
    qkv_chunk(kT_sb[0], 0, 0, 0)
    emit_sub(0, 0, [
        F(qkv_chunk, kT_sb[0], 0, 0, 1), None, None,
        F(qkv_chunk, kT_sb[0], 0, 0, 2), None, None,
        F(qkv_chunk, kT_sb[0], 0, 0, 3), None,
        F(qkv_chunk, vT[0], 0, 2, 0), None,
        F(qkv_chunk, vT[0], 0, 2, 1), None,
        F(emit_vtr, 0, [0, 1]), None,
        F(emit_vtr, 0, [2, 3]), None])
    emit_sub(0, 1, [
        F(qkv_chunk, vT[0], 0, 2, 2), None,
        F(qkv_chunk, vT[0], 0, 2, 3), None,
        F(emit_vtr, 0, [4, 5]), F(emit_vtr, 0, [6, 7]),
        F(emit_vtr, 0, [8, 9]), F(emit_vtr, 0, [10, 11]),
        F(emit_vtr, 0, [12, 13]), F(emit_vtr, 0, [14, 15])])
    emit_sub(0, 2, [
        F(emit_attend, 0, 0, 0), None, None, None, None, None, None, None,
        F(emit_attend, 0, 0, 1), None, None, None,
        F(qkv_chunk, kT_sb[1], 1, 0, 0), None,
        F(qkv_chunk, kT_sb[1], 1, 0, 1), None])
    emit_sub(0, 3, [
        F(emit_attend, 0, 1, 0), None, None, None, None, None, None, None,
        F(emit_attend, 0, 1, 1), None, None, None,
        F(qkv_chunk, kT_sb[1], 1, 0, 2), None,
        F(qkv_chunk, kT_sb[1], 1, 0, 3), None])
    emit_sub(1, 0, [
        F(emit_attend, 0, 2, 0), None, None, None, None, None, None, None,
        F(emit_attend, 0, 2, 1), None, None, None,
        F(qkv_chunk, vT[1], 1, 2, 0), None,
        F(qkv_chunk, vT[1], 1, 2, 1), None])
    emit_sub(1, 1, [
        F(emit_attend, 0, 3, 0), None, None, None, None, None, None, None,
        F(emit_attend, 0, 3, 1), None, None,
        F(qkv_chunk, vT[1], 1, 2, 2),
        F(qkv_chunk, vT[1], 1, 2, 3),
        F(emit_vtr, 1, [0, 1, 2, 3]),
        F(emit_vtr, 1, [4, 5, 6, 7]),
        F(emit_vtr, 1, [8, 9, 10, 11])])
    emit_sub(1, 2, [
        F(emit_vtr, 1, [12, 13, 14, 15]), None,
        F(emit_attend, 1, 0, 0), None, None, None, None, None,
        F(emit_attend, 1, 0, 1), None, None, None,
        F(emit_outproj_lt, 0), F(emit_outproj_lt, 1),
        F(emit_outproj_lt, 2), F(emit_outproj_lt, 3)])
    emit_sub(1, 3, [
        F(emit_attend, 1, 1, 0), None, None, None,
        F(emit_attend, 1, 1, 1), None, None, None,
        F(emit_attend, 1, 2, 0), None,
        F(emit_attend_p1, 1, 3, 0), None,
        F(emit_attend, 1, 2, 1), None,
        F(emit_attend_p1, 1, 3, 1),
        F(emit_outproj_lt, 4)])
    for lt in range(5, 12):
        emit_outproj_lt(lt, wide=(lt >= 8))
    emit_attend(1, 3, 0)
    emit_attend(1, 3, 1)
    for lt in range(12, 16):
        emit_outproj_lt(lt, wide=True)


_NC_CACHE = {}


def _get_nc():
    if "nc" not in _NC_CACHE:
        _NC_CACHE["nc"] = build_nc()
    return _NC_CACHE["nc"]


def shard_inputs(x, Wq, bq, Wk, bk, Wv, bv, Wd, bd):
    """Build the 8 per-core input maps (host picks the on-device layout)."""
    in_maps = []
    x = np.asarray(x, np.float32)
    for c in range(NCORES):
        b = c // (NCORES // B)
        h0 = (c % (NCORES // B)) * H_PER_CORE
        hs = slice(h0, h0 + H_PER_CORE)
        # x packed [k, lc, kt, l'] bf16: contiguous 8KB DMA lines per chunk
        xT = (x[b].T.reshape(KT, P, NSUB, LCH).transpose(1, 2, 0, 3)
              .reshape(P, -1).astype(BF16_NP))
        xT = np.ascontiguousarray(xT)
        # wqkv bf16 [128, pp * KQV * kt * 128]: [k, pp, i, kt, n]
        ws = []
        for W in (Wk, Wq, Wv):
            w = np.asarray(W[:, hs, :], np.float32).reshape(DMODEL, 2 * P)
            ws.append(w.reshape(KT, P, NPAIR, P).transpose(1, 2, 0, 3))
        wqkv = np.stack(ws, axis=2).reshape(P, -1).astype(BF16_NP)
        # wd2 bf16 [128, pp * dmodel]: [k, pp, m]
        wd2 = (np.asarray(Wd[hs], np.float32).reshape(NPAIR, P, DMODEL)
               .transpose(1, 0, 2).reshape(P, -1).astype(BF16_NP))
        # biasp f32 [128, KQV * pp]: [k, i, pp]
        bs = [np.asarray(v[hs], np.float32).reshape(NPAIR, P).T
              for v in (bk, bq, bv)]
        biasp = np.ascontiguousarray(
            np.stack(bs, axis=1).reshape(P, -1))
        in_maps.append({
            "x": xT,
            "wqkv": np.ascontiguousarray(wqkv),
            "wd2": np.ascontiguousarray(wd2),
            "biasp": biasp,
        })
    return in_maps


def gather_outputs(results, bd):
    """Sum partial outputs per batch and add bd."""
    out = np.zeros((B, L, DMODEL), np.float32)
    per_b = NCORES // B
    for c, res in enumerate(results):
        out[c // per_b] += np.asarray(res["y"], np.float32)
    out += np.asarray(bd, np.float32)[None, None, :]
    return out


def kernel(x, Wq, bq, Wk, bk, Wv, bv, Wd, bd, _trace=False):
    nc = _get_nc()
    in_maps = shard_inputs(x, Wq, bq, Wk, bk, Wv, bv, Wd, bd)
    res = run_bass_kernel_spmd(nc, in_maps, list(range(NCORES)), trace=_trace)
    out = gather_outputs(res.results, bd)
    if _trace:
        kernel.last_results = res
    return out
